# revision 49
# baseline (speedup 1.0000x reference)
"""ALiBi causal attention (B=2, T=2048, D=1024, H=16) on 8 TRN2 NeuronCores.

Sharding: tensor-parallel over heads, 2 heads per core, zero collectives;
the host sums the 8 full-width partial outputs.

v2 layout: per-head causal windows. ALiBi decay means head h only needs
keys within W_h = SAFE/slope_h; heads are grouped by window size into the
graph's four (col-half, slot) attention structures:
  half0 slotA: C=16 chunks (full causal)  <- heads {12,13,14,15}
  half0 slotB: C=7                        <- heads {8,9,10,11}
  half1 slotA: C=3                        <- heads {4,5,6,7}
  half1 slotB: C=2                        <- heads {0,1,2,3}
SPMD runs the same structure on all cores; which (head, batch) task a
core's (half, slot) runs is pure input data: cores 0-3 put batch 0 in
column-half 0, cores 4-7 put batch 1 there, so each group of four cores
covers one batch of each head group. The host un-permutes when summing.

Kernel-internal notes:
- scores are computed transposed, sT[k, q]; the exact ALiBi bias is folded
  into the QK matmul via 4 extra contraction features (split-precision bf16
  pair). Scores for consecutive (ki, qh) windows are PACKED into shared
  [128,1024] PSUM tiles so one ACT exp covers up to 1024 columns (the
  352-cycle ACT pipe fill dominated per-window exps for narrow windows).
- v3 carries 64 ones-columns per slot, so every AV matmul broadcasts the
  softmax denominator into acc rows 64-127 for free; normalization is
  ACT ln + exp(-x) + one DVE multiply, per 512-column group as soon as
  that group's AVs have flushed (lets out-mix tiles spread through the
  later slots instead of bunching in the tail).
- the PE HAM clock gate (1.2 GHz cold / 2.4 GHz warm, ~3.4us windows) is
  the dominant effect to manage: a warm-up spinner covers the initial DMA
  dead zone, chunks 4-7 of the projection interleave into the first
  attention slot's ACT-limited stretches, and out-mix tiles fill PE idle
  slots so the gate stays open.
- input DMA is aggregate-bandwidth-bound early on: x streams on the
  GpSimd hwdge queue, xt 0-3 on ACT's, xt 4-7 + weights on SP's; chunk
  4-7 issues are emitted lazily from inside proj tasks so their pool-slot
  waits never wedge a queue in front of the attention tri-masks (GpSimd).
- k-side projection bias is dropped entirely: softmax over keys is
  invariant to it ((q+bq)o(k+bk): the q.bk and bq.bk terms are constant
  per query). q keeps its bias, applied on ACT during the PSUM->SBUF copy.
- all DRAM inputs are laid out so every DMA is a full-partition contiguous
  block (8KB runs), not a strided gather.
"""
import math
from contextlib import ExitStack

import numpy as np
import ml_dtypes

import concourse.bass as bass
import concourse.tile as tile
import concourse.mybir as mybir
from concourse.bass_utils import run_bass_kernel_spmd
from concourse.masks import make_identity

B, T, D, H = 2, 2048, 1024, 16
HD = D // H          # 64
BT = B * T           # 4096
NCORES = 8
SHIFT = 6.0          # uniform score shift inside exp (cancels in softmax);
                     # small enough that ln(den) stays O(1) in magnitude
# chunks kept per (col-half, slot): nb = chunks behind the diagonal
# SAFE=8 e-folds of ALiBi decay: dropped-key mass <= e^-8 of kept mass.
NB = ((15, 4), (1, 1))
# head groups per (half, slot): heads listed by c%4
HEADS = (((12, 13, 14, 15), (8, 9, 10, 11)), ((4, 5, 6, 7), (0, 1, 2, 3)))
BF = mybir.dt.bfloat16
F32 = mybir.dt.float32
BF_NP = ml_dtypes.bfloat16


def _alibi_slopes(n_heads):
    def pow2_slopes(n):
        start = 2.0 ** (-(2.0 ** (-(math.log2(n) - 3))))
        return [start * (start ** i) for i in range(n)]
    if n_heads & (n_heads - 1) == 0:
        slopes = pow2_slopes(n_heads)
    else:
        c = 2 ** math.floor(math.log2(n_heads))
        slopes = pow2_slopes(c)
        extra_base = 2.0 ** (-(2.0 ** (-(math.log2(2 * c) - 3))))
        slopes += [extra_base * (extra_base ** i) for i in range(n_heads - c)]
    return np.asarray(slopes[:n_heads], dtype=np.float32)


WAIT_LIMITS = {"InstDrain": 1, "InstEventSemaphore": 1, "default": 1}


def split_sync_waits(nc):
    """Walrus caps sync-wait conditions per instruction (per ISA struct) at 1.
    Excess waits are hoisted onto preceding same-engine instructions with a
    free wait slot (waiting earlier on an in-order engine is always safe);
    drains are inserted only when no host instruction is available (drains
    flush the engine pipe, which hurts PE back-to-back throughput)."""
    n_hoist = n_drain = 0
    skip = {"InstRegisterMove", "InstUnconditionalBranch", "InstCall",
            "InstISA"}
    for f in nc.m.functions:
        for bb in f.blocks:
            insts = bb.instructions
            i = 0
            while i < len(insts):
                inst = insts[i]
                si = inst.sync_info
                limit = WAIT_LIMITS.get(
                    type(inst).__name__, WAIT_LIMITS["default"])
                if si is not None and si.on_wait and len(si.on_wait) > limit:
                    waits = list(si.on_wait)
                    # Put long-latency (cross-engine) waits on the carrier
                    # drains — their pipe-flush overlaps the sem wait — and
                    # keep same-engine waits (usually already satisfied) on
                    # the instruction itself.
                    eng = str(inst.engine).split(".")[-1]
                    pfx = {"Activation": "Activation", "DVE": "DVE",
                           "PE": "PE", "Pool": "Pool", "SP": "Sync"}.get(
                        eng, "\x00")
                    waits.sort(key=lambda w: 0 if str(
                        w.ant_name or "").startswith(pfx) else 1)
                    excess, keep = waits[limit:], waits[:limit]
                    inst.sync_info = mybir.SyncInfo(
                        on_wait=keep, on_update=list(si.on_update or [])
                    )
                    # Hoist onto preceding same-engine insts with a free wait
                    # slot. Never scan past a same-engine instruction that
                    # carries an on_update: anything another engine could be
                    # waiting on (and that our waited-sem's producer chain
                    # might depend on) is signalled via such an update, so
                    # stopping there makes the early-wait deadlock-free.
                    j = i - 1
                    lim = max(0, i - 24)
                    while excess and j >= lim:
                        p = insts[j]
                        if p.engine == inst.engine:
                            psi = p.sync_info
                            if psi is not None and psi.on_update:
                                break
                            if type(p).__name__ not in skip and (
                                psi is None or not psi.on_wait
                            ):
                                p.sync_info = mybir.SyncInfo(
                                    on_wait=[excess.pop()], on_update=[])
                                n_hoist += 1
                        j -= 1
                    off = 0
                    for w in excess:
                        nop = mybir.InstDrain(
                            name=f"{inst.name}-wsplit{off}", ins=[], outs=[]
                        )
                        nop.engine = inst.engine
                        nop.sync_info = mybir.SyncInfo(on_wait=[w], on_update=[])
                        insts.insert(i + off, nop)
                        off += 1
                        n_drain += 1
                    i += off
                i += 1
    return n_hoist, n_drain


def build_graph():
    nc = bass.Bass()
    # Walrus rejects EVENT_SEMAPHORE_RANGE_CLEAR over wide ranges
    # ("ISA wrong length"); chunk the kernel-tail sem clear.
    orig_clear = nc.clear_and_free_semaphores

    def chunked_clear(sems):
        sems = sorted(
            s.num if hasattr(s, "num") else s for s in sems)
        for i in range(0, len(sems), 8):
            orig_clear(sems[i:i + 8])

    nc.clear_and_free_semaphores = chunked_clear
    dp = nc.declare_dram_parameter
    xT = dp("xT", [8, 128, 4096], BF, isOutput=False)
    xtT = dp("xtT", [8, 128, 4096], BF, isOutput=False)
    wq = dp("wq", [128, 2048], BF, isOutput=False)
    wk = dp("wk", [128, 2048], BF, isOutput=False)
    bq = dp("bq", [128, 2], F32, isOutput=False)
    mv = dp("mv", [128, 2048], BF, isOutput=False)
    mo = dp("mo", [128, 2048], BF, isOutput=False)
    qaug = dp("qaug", [2, 4, BT], BF, isOutput=False)
    kaug = dp("kaug", [2, 4, BT], BF, isOutput=False)
    tri = dp("tri", [128, 128], BF, isOutput=False)
    out_ext = dp("out", [BT, D], BF, isOutput=True)

    with tile.TileContext(nc) as tc, ExitStack() as ctx:
        persist = ctx.enter_context(tc.tile_pool(name="persist", bufs=1))
        xs_v = ctx.enter_context(tc.tile_pool(name="xs_v", bufs=4))
        xs_p = ctx.enter_context(tc.tile_pool(name="xs_p", bufs=4))
        vstage = ctx.enter_context(tc.tile_pool(name="vstage", bufs=8))
        expp = ctx.enter_context(tc.tile_pool(name="expp", bufs=10))
        outp = ctx.enter_context(tc.tile_pool(name="outp", bufs=6))
        recp = ctx.enter_context(tc.tile_pool(name="recp", bufs=2))
        psp = ctx.enter_context(tc.tile_pool(name="psp", bufs=2, space="PSUM"))
        psacc = ctx.enter_context(tc.tile_pool(name="psacc", bufs=2, space="PSUM"))

        # ---- persistent tiles ----
        wq_sb = persist.tile([128, 2048], BF, tag="wq_sb")
        wk_sb = persist.tile([128, 2048], BF, tag="wk_sb")
        mv_sb = persist.tile([128, 2048], BF, tag="mv_sb")
        mo_sb = persist.tile([128, 2048], BF, tag="mo_sb")
        bq_sb = persist.tile([128, 2], F32, tag="bq_sb")
        tri_sb = persist.tile([128, 128], BF, tag="tri_sb")
        ident = persist.tile([128, 128], BF, tag="ident")
        # v layout per (kt, slot): cols 0-63 data, 64-127 ones. The ones
        # columns make every AV accumulate 64 copies of the denominator
        # row into acc[64:128] -- a free cross-partition broadcast on PE,
        # so normalization is just DVE reciprocal + multiply.
        v_sb = persist.tile([128, 32 * 256], BF, tag="v_sb")
        z_sb = persist.tile([128, BT], BF, tag="z_sb")
        qk_sb = {}
        for slot in range(2):
            qk_sb[("q", slot)] = persist.tile([68, BT], BF, tag=f"q{slot}_sb", name=f"q{slot}_sb")
            qk_sb[("k", slot)] = persist.tile([68, BT], BF, tag=f"k{slot}_sb", name=f"k{slot}_sb")

        # weight/const loads in consumption order. tri/aug are needed by
        # the very first attention scores (~12us in); mo only by the first
        # out-mix (~50us), so it goes last.
        nc.sync.dma_start(wq_sb[:], wq[:])
        nc.sync.dma_start(wk_sb[:], wk[:])
        nc.sync.dma_start(bq_sb[:], bq[:])
        nc.sync.dma_start(mv_sb[:], mv[:])
        nc.sync.dma_start(tri_sb[:], tri[:])
        for slot in range(2):
            nc.sync.dma_start(qk_sb[("q", slot)][64:68, :], qaug[slot])
            nc.sync.dma_start(qk_sb[("k", slot)][64:68, :], kaug[slot])
        nc.sync.dma_start(mo_sb[:], mo[:])
        make_identity(nc, ident[:])
        shift_sb = persist.tile([128, 1], F32, tag="shift_sb")
        nc.vector.memset(shift_sb[:], -SHIFT)
        # preload the ACT exp table set during the DMA dead zone so the
        # first real exp doesn't stall ~2.7us mid-kernel
        warm_act = persist.tile([1, 1], F32, tag="warm_act")
        nc.scalar.activation(warm_act[:], shift_sb[0:1, 0:1],
                             mybir.ActivationFunctionType.Exp)
        v3 = v_sb[:].rearrange("p (t c) -> p t c", c=256)
        nc.vector.memset(v3[:, :, 64:128], 1.0)
        nc.vector.memset(v3[:, :, 192:256], 1.0)

        # ---- HAM warm-up: the PE clock gate defaults to 1.2 GHz and only
        # reaches 2.4 GHz after ~3.4us of sustained matmul activity. Spin
        # dummy transposes through a scratch PSUM bank while the first
        # x/xt chunks stream in, so the real projection starts warm.
        # 72 spins ~= 5-8us: enough that the gate opens (~3.4us sustained)
        # and stays open until the first x/weight DMAs land, so the whole
        # DMA-ramp phase (projection + first attention pass) runs at 2.4
        # GHz instead of 1.2.
        warm_ps = psp.tile([128, 128], BF, tag="ps", name="warm_ps")
        for _ in range(72):
            nc.tensor.transpose(warm_ps[:], ident[:], ident[:])

        # ---- input streams: x on the GpSimd hwdge queue; xt 0-3 on the
        # ACT queue (free pool slots, so they issue immediately and in
        # parallel with the weights on SP), xt 4-7 on SP behind the
        # weights. Each hwdge queue is descriptor-rate-limited to ~250-330
        # GB/s; three queues together reach the HBM roofline instead of
        # starving the projection on one.
        # Only chunks 0-1 issue up front: the aggregate DMA bandwidth
        # (~350 GB/s over all queues) is the binding constraint early on,
        # so issuing everything immediately just dilutes the bytes the
        # first projection actually waits for. Later chunks issue lazily
        # (lead 2) from inside the proj tasks, which also keeps their
        # pool-slot waits from wedging a queue in front of other work.
        xch = {}

        def issue_xdma(ch):
            x_t = xs_p.tile([128, 4096], BF, tag="x_t", name="x_t")
            nc.gpsimd.dma_start(x_t[:], xT[ch])
            xt_t = xs_v.tile([128, 4096], BF, tag="xt_t", name="xt_t")
            (nc.scalar if ch < 4 else nc.sync).dma_start(xt_t[:], xtT[ch])
            xch[ch] = (x_t, xt_t)

        for ch in range(4):
            issue_xdma(ch)
        # attention-phase constants queue after the xt stream on SP
        nc.sync.dma_start(mo_sb[:], mo[:])
        nc.sync.dma_start(tri_sb[:], tri[:])
        for slot in range(2):
            nc.sync.dma_start(qk_sb[("q", slot)][64:68, :], qaug[slot])
            nc.sync.dma_start(qk_sb[("k", slot)][64:68, :], kaug[slot])

        # ---- projection pieces as schedulable tasks. Chunks 0-3 (column
        # half 0) run up front; chunks 4-7 interleave into the first
        # attention slot's ACT-limited stretches so the PE never idles
        # long enough for the HAM clock gate to re-throttle.
        vt_map = {}

        def proj_qk(ch, part):
            bg = ch // 4
            if part == "q" and 4 <= ch + 2 < 8:
                issue_xdma(ch + 2)
            w_sb = wq_sb if part == "q" else wk_sb
            cols = bass.ts(ch, 512)
            ps_p = psp.tile([128, 512], F32, tag="ps")
            for s in range(8):
                nc.tensor.matmul(
                    ps_p[:], w_sb[:, bg * 1024 + s * 128:bg * 1024 + (s + 1) * 128],
                    xch[ch][0][:, s * 512:(s + 1) * 512],
                    start=(s == 0), stop=(s == 7),
                )
            for slot in range(2):
                pr = ps_p[slot * 64:(slot + 1) * 64, :]
                dst = qk_sb[(part, slot)][0:64, cols]
                if part == "q":
                    nc.scalar.activation(
                        dst, pr, mybir.ActivationFunctionType.Identity,
                        bias=bq_sb[slot * 64:(slot + 1) * 64, bg:bg + 1])
                else:
                    # k needs no bias (softmax is invariant to it); plain
                    # copies go on DVE to keep ACT free for exps
                    nc.vector.tensor_copy(dst, pr)

        def proj_v(ch):
            bg = ch // 4
            ps_vt = psp.tile([128, 512], F32, tag="ps")
            for s in range(8):
                nc.tensor.matmul(
                    ps_vt[:], mv_sb[:, bg * 1024 + s * 128:bg * 1024 + (s + 1) * 128],
                    xch[ch][1][:, s * 512:(s + 1) * 512],
                    start=(s == 0), stop=(s == 7),
                )
            vt_sb = vstage.tile([128, 512], BF, tag="vt_sb",
                                name=f"vt_sb{ch}")
            nc.vector.tensor_copy(vt_sb[:], ps_vt[:])
            vt_map[ch] = vt_sb

        def proj_tr(ch):
            # transpose vT -> v[k, (slot,d)]: 4 transposes into one PSUM
            # tile, two strided DVE copies (per slot group) into v3
            ps_tr = psp.tile([128, 512], BF, tag="ps", name="ps_tr")
            for q in range(4):
                nc.tensor.transpose(ps_tr[:, q * 128:(q + 1) * 128],
                                    vt_map[ch][:, q * 128:(q + 1) * 128],
                                    ident[:])
            for g in range(2):
                nc.vector.tensor_copy(
                    v3[:, ch * 4:ch * 4 + 4, g * 128:g * 128 + 64],
                    ps_tr[:].rearrange("p (t g c) -> p t g c", g=2, c=64)
                    [:, :, g, :])

        for ch in range(2):
            proj_qk(ch, "q")
            proj_qk(ch, "k")
            proj_v(ch)
            proj_tr(ch)
        proj_tasks = []
        for ch in range(2, 8):
            proj_tasks.append(lambda ch=ch: proj_qk(ch, "q"))
            proj_tasks.append(lambda ch=ch: proj_qk(ch, "k"))
            proj_tasks.append(lambda ch=ch: proj_v(ch))
            proj_tasks.append(lambda ch=ch: proj_tr(ch))
        proj_tasks = list(reversed(proj_tasks))  # pop() from the front

        # ---- attention + out-mix ----
        # Scores for consecutive (ki, qh) windows are PACKED into shared
        # [128,1024] PSUM tiles so one ACT exp covers up to 1024 columns
        # (the 352-cycle ACT pipe fill made per-window exps the pacer for
        # the narrow-window slots). yT' accumulates in [128,1024] q-halves;
        # rows 64-127 all carry the denominator via the ones columns of v3
        # (free PE broadcast), so normalization is ACT ln/exp + DVE mult.
        # Norms are emitted per 512-column group as soon as that group's
        # AVs have flushed, which lets out-mix tiles spread through the
        # later slots instead of bunching in the tail.
        group_marks = {}
        outmix_ready = []

        def mark_norm(b_, g8):
            group_marks[(b_, g8)] = group_marks.get((b_, g8), 0) + 1
            if group_marks[(b_, g8)] == 2:
                outmix_ready.extend((b_, qt) for qt in range(g8 * 4, g8 * 4 + 4))

        omix_n = [0]

        def emit_outmix(bq_, qt):
            ps_o = psp.tile([128, 1024], F32, tag="ps", name="ps_o")
            zc = z_sb[:, bq_ * T + qt * 128:bq_ * T + (qt + 1) * 128]
            for piece in range(2):
                nc.tensor.matmul(
                    ps_o[:, piece * 512:(piece + 1) * 512],
                    zc, mo_sb[:, bq_ * 1024 + piece * 512:bq_ * 1024 + (piece + 1) * 512],
                    start=True, stop=True,
                )
            o_sb = outp.tile([128, 1024], BF, tag="o_sb", name="o_sb")
            # the PSUM->SBUF casts are the largest single DVE cost (1.2us
            # each at 1x rate); alternate them onto ACT to split the load
            omix_n[0] += 1
            if omix_n[0] % 2:
                nc.vector.tensor_copy(o_sb[:], ps_o[:])
            else:
                nc.scalar.copy(o_sb[:], ps_o[:])
            nc.sync.dma_start(
                out_ext[(bq_ * 16 + qt) * 128:(bq_ * 16 + qt + 1) * 128, :],
                o_sb[:])

        for b in range(2):
            base = b * T
            for slot in range(2):
                nb = NB[b][slot]
                q_t, k_t = qk_sb[("q", slot)], qk_sb[("k", slot)]
                # norm granularity: 512 cols so z is finalized (and the
                # slot-boundary norm chain shortened) as early as possible
                ng = 512
                accs = []
                for qh in range(2):
                    acc = psacc.tile([128, 1024], F32, tag="acc",
                                     name=f"acc{qh}")
                    nc.vector.memset(acc[:], 0.0)
                    accs.append(acc)

                # --- score packer state ---
                pk = {"ps": None, "c": 0, "segs": [], "n": 0}
                avq = []          # (qh, ki, expT, c0, lo, hi, packid)
                flushed_ki = [-1, -1]
                norm_next = [0, 0]      # next ng-group to normalize, per qh
                pend_mult = []

                def close_pack():
                    if pk["ps"] is None:
                        return
                    w = pk["c"]
                    expT = expp.tile([128, 1024], BF, tag="expT")
                    nc.scalar.activation(
                        expT[:, 0:w], pk["ps"][:, 0:w],
                        mybir.ActivationFunctionType.Exp, bias=shift_sb[:])
                    for (ki, qh, lo, hi, c0) in pk["segs"]:
                        dhi = min(hi, ki * 128 + 128)
                        if lo < dhi:
                            # causal mask of the (possibly split) diagonal
                            # block: multiplicative upper-tri zeroing on
                            # GpSimd -- the only engine with slack here
                            nc.gpsimd.tensor_mul(
                                expT[:, c0:c0 + (dhi - lo)],
                                expT[:, c0:c0 + (dhi - lo)],
                                tri_sb[:, lo - ki * 128:dhi - ki * 128])
                        avq.append((qh, ki, expT, c0, lo, hi, pk["n"]))
                    pk["ps"] = None
                    pk["c"] = 0
                    pk["segs"] = []
                    pk["n"] += 1

                def emit_seg(ki, qh, lo, hi):
                    kc = k_t[:, base + ki * 128:base + ki * 128 + 128]
                    while lo < hi:
                        if pk["ps"] is None:
                            pk["ps"] = psp.tile([128, 1024], F32, tag="ps",
                                                name="pk_ps")
                        c0 = pk["c"]
                        w = min(hi - lo, 1024 - c0)
                        c = c0
                        while c < c0 + w:
                            pw = min(c0 + w, (c // 512 + 1) * 512) - c
                            nc.tensor.matmul(
                                pk["ps"][:, c:c + pw],
                                kc,
                                q_t[:, base + lo + (c - c0):base + lo + (c - c0) + pw],
                                start=True, stop=True,
                            )
                            c += pw
                        pk["segs"].append((ki, qh, lo, lo + w, c0))
                        pk["c"] = c0 + w
                        lo += w
                        if pk["c"] == 1024:
                            close_pack()

                def flush_av(accs=accs, slot=slot, b=b):
                    qh, ki, expT, c0, s_lo, s_hi, _ = avq.pop(0)
                    qlo = qh * 1024
                    a = s_lo
                    while a < s_hi:
                        nxt = min(s_hi, ((a - qlo) // 512 + 1) * 512 + qlo)
                        nc.tensor.matmul(
                            accs[qh][0:128, a - qlo:nxt - qlo],
                            v3[:, b * 16 + ki, slot * 128:slot * 128 + 128],
                            expT[:, c0 + (a - s_lo):c0 + (nxt - s_lo)],
                            start=False, stop=False, skip_group_check=True,
                        )
                        a = nxt
                    flushed_ki[qh] = max(flushed_ki[qh], ki)

                def flush_mult(b=b):
                    qh, alo, rec, slot_, zlo = pend_mult.pop(0)
                    gw = ng
                    nc.vector.tensor_mul(
                        z_sb[slot_ * 64:(slot_ + 1) * 64, zlo:zlo + gw],
                        accs[qh][0:64, alo:alo + gw],
                        rec)
                    # out-mix readiness: only now is the z region written
                    # (marking at norm-queue time would let an out-mix read
                    # stale z -- the multiply is deferred)
                    for g8 in range((qh * 1024 + alo) // 512,
                                    (qh * 1024 + alo + gw) // 512):
                        mark_norm(b, g8)

                def try_norms(slot=slot, b=b):
                    for qh in range(2):
                        while norm_next[qh] * ng < 1024:
                            g = norm_next[qh]
                            last_ki = qh * 8 + (g + 1) * (ng // 128) - 1
                            if flushed_ki[qh] < last_ki:
                                break
                            alo = g * ng
                            lnd = recp.tile([64, 1024], F32, tag="lnd",
                                            name="lnd")
                            nc.scalar.activation(
                                lnd[:, 0:ng], accs[qh][64:128, alo:alo + ng],
                                mybir.ActivationFunctionType.Ln)
                            rec = recp.tile([64, 1024], F32, tag="rec",
                                            name="rec")
                            nc.scalar.activation(
                                rec[:, 0:ng], lnd[:, 0:ng],
                                mybir.ActivationFunctionType.Exp, scale=-1.0)
                            pend_mult.append(
                                (qh, alo, rec[:, 0:ng], slot,
                                 base + qh * 1024 + alo))
                            norm_next[qh] += 1
                    # flush all but the freshest deferred z-multiply: by the
                    # next call its recip has cleared the ACT queue, so the
                    # DVE never stalls (a stalled DVE blocks later tri masks
                    # and with them the PE's AV stream)
                    while len(pend_mult) > 1:
                        flush_mult()

                def deferred_work(budget, proj_ok=True):
                    # Interleave proj pieces / out-mix tiles into the PE
                    # stream. Each allocates a psp tile, which is only
                    # deadlock-free while no score pack is open (an open
                    # pack's slot must not be claimed by a later alloc
                    # whose own MMs would then precede the pack's exp).
                    while budget > 0 and pk["ps"] is None and (
                            outmix_ready or (proj_ok and proj_tasks)):
                        if outmix_ready:
                            emit_outmix(*outmix_ready.pop(0))
                        else:
                            proj_tasks.pop()()
                        budget -= 1

                if (b, slot) == (0, 0):
                    # two sequential q-half passes: the qh0 pass only needs
                    # token chunks 0-1 projected, so attention starts while
                    # the input stream is still ramping; chunks 2-3 project
                    # inside the qh0 pass, 4-7 inside the qh1 pass
                    passes = [((0,), range(0, 8)), ((1,), range(0, 16))]
                else:
                    passes = [((0, 1), range(0, 16))]
                for pidx, (qhs, kis) in enumerate(passes):
                    for ki in kis:
                        # gate proj pops so an early-popped proj MM never
                        # sits at the head of the PE queue waiting for its
                        # (DMA-bound) input while ready scores queue behind
                        pok = True
                        for qh in qhs:
                            qlo, qhi = qh * 1024, (qh + 1) * 1024
                            s_lo = max(qlo, ki * 128)
                            s_hi = min(qhi, (ki + nb + 1) * 128)
                            if s_lo < s_hi:
                                emit_seg(ki, qh, s_lo, s_hi)
                                deferred_work(1, pok)
                        # AVs lag one closed pack behind the open one
                        while avq and avq[0][6] < pk["n"] - 1:
                            flush_av()
                        try_norms()
                        # near-full pack + starving deferred queue: close
                        # early so the deferred work can use the pool
                        if pk["c"] >= 768 and (
                                len(proj_tasks) + len(outmix_ready) > 2):
                            close_pack()
                        deferred_work(2, pok)
                    if (b, slot) == (0, 0) and len(proj_tasks) > 16:
                        # qh1 scores need chunks 2-3 fully projected
                        close_pack()
                        while len(proj_tasks) > 16:
                            proj_tasks.pop()()
                # slot end: drain everything so the accs can rotate
                close_pack()
                while avq:
                    flush_av()
                try_norms()
                while pend_mult:
                    flush_mult()
                deferred_work(6)
            if b == 0:
                # half-1 attention needs chunks 4-7 fully projected
                while proj_tasks:
                    proj_tasks.pop()()

        while outmix_ready:
            emit_outmix(*outmix_ready.pop(0))

    return nc


def _bf16_split(x):
    hi = x.astype(BF_NP)
    lo = (x - hi.astype(np.float32)).astype(BF_NP)
    return hi, lo


def _wt_layout(w):
    """[128 out, 1024 in] row-major weights -> PE stationary layout
    [128 part(K within slab), 8 slabs * 128 M] contiguous per partition."""
    return np.ascontiguousarray(
        w.T.reshape(8, 128, 128).transpose(1, 0, 2).reshape(128, 1024))


def make_in_maps(x_norm, xt, qk_w, qk_b, v_fact, out_fact):
    slopes = _alibi_slopes(H)
    scale = 1.0 / math.sqrt(HD)
    pos = np.tile(np.arange(T, dtype=np.float32), B)
    # multiplicative causal mask for the diagonal 128x128 block, applied
    # to the exp'd scores on DVE: 1 where k <= q else 0
    tri = np.where(np.arange(128)[:, None] <= np.arange(128)[None, :],
                   1.0, 0.0).astype(BF_NP)

    xr = x_norm.reshape(BT, D)
    xtr = xt.reshape(BT, D)

    def chunked_xT(x2):
        # x2: [1024 D, 4096 tokens] -> [8 ch, 128 p, 8 s * 512] where
        # partition p of chunk ch holds slabs s contiguously
        return np.ascontiguousarray(
            x2.reshape(8, 128, 8, 512).transpose(2, 1, 0, 3).reshape(
                8, 128, 4096)).astype(BF_NP)

    in_maps = []
    for c in range(NCORES):
        g = c % 4
        b0 = 0 if c < 4 else 1          # batch living in column-half 0
        halves = (b0, 1 - b0)
        # token permutation: col-half hf holds batch halves[hf]
        tok = np.concatenate([np.arange(T) + halves[0] * T,
                              np.arange(T) + halves[1] * T])
        x2 = np.ascontiguousarray(xr[tok].T)     # [D, 4096] permuted
        xt2 = np.ascontiguousarray(xtr[tok].T)

        # per (half, slot) head
        hmap = [[HEADS[hf][s][g] for s in range(2)] for hf in range(2)]

        wq_c = np.zeros((128, 2048), np.float32)
        wk_c = np.zeros((128, 2048), np.float32)
        bq_c = np.zeros((128, 2), np.float32)
        mv_c = np.zeros((128, 2048), np.float32)
        mo_c = np.zeros((128, 2048), np.float32)
        qaug_c = np.zeros((2, 4, BT), np.float32)
        kaug_c = np.zeros((2, 4, BT), np.float32)
        dd = np.arange(64)
        for hf in range(2):
            rq = np.concatenate(
                [qk_w[h * HD:(h + 1) * HD] for h in hmap[hf]]) * scale
            rk = np.concatenate(
                [qk_w[D + h * HD:D + (h + 1) * HD] for h in hmap[hf]])
            wq_c[:, hf * 1024:(hf + 1) * 1024] = _wt_layout(rq)
            wk_c[:, hf * 1024:(hf + 1) * 1024] = _wt_layout(rk)
            bq_c[:, hf] = np.concatenate(
                [qk_b[h * HD:(h + 1) * HD] for h in hmap[hf]]) * scale

            mvk = np.zeros((16, 64, 2, 64), np.float32)
            mok = np.zeros((2, 64, 16, 64), np.float32)
            for jl, h in enumerate(hmap[hf]):
                for m in range(16):
                    mvk[m, dd, jl, dd] = v_fact[h, m]
                for i in range(16):
                    mok[jl, dd, i, dd] = out_fact[i, h]
            mv_c[:, hf * 1024:(hf + 1) * 1024] = _wt_layout(
                mvk.reshape(1024, 128).T)
            mo_c[:, hf * 1024:(hf + 1) * 1024] = mok.reshape(128, 1024)

            for jl, h in enumerate(hmap[hf]):
                ab = slopes[h] * pos[hf * T:(hf + 1) * T]
                hi, lo = _bf16_split(ab)
                cs = slice(hf * T, (hf + 1) * T)
                qaug_c[jl, 0, cs] = -hi.astype(np.float32)
                qaug_c[jl, 1, cs] = -lo.astype(np.float32)
                qaug_c[jl, 2, cs] = 1.0
                qaug_c[jl, 3, cs] = 1.0
                kaug_c[jl, 0, cs] = 1.0
                kaug_c[jl, 1, cs] = 1.0
                kaug_c[jl, 2, cs] = hi.astype(np.float32)
                kaug_c[jl, 3, cs] = lo.astype(np.float32)

        in_maps.append({
            "xT": chunked_xT(x2), "xtT": chunked_xT(xt2),
            "wq": wq_c.astype(BF_NP), "wk": wk_c.astype(BF_NP),
            "bq": bq_c,
            "mv": mv_c.astype(BF_NP), "mo": mo_c.astype(BF_NP),
            "qaug": qaug_c.astype(BF_NP), "kaug": kaug_c.astype(BF_NP),
            "tri": tri,
        })
    return in_maps


_GRAPH = None


def _get_graph():
    global _GRAPH
    if _GRAPH is None:
        _GRAPH = build_graph()
        split_sync_waits(_GRAPH)
    return _GRAPH


def run(in_maps, **kw):
    nc = _get_graph()
    return run_bass_kernel_spmd(nc, in_maps, list(range(NCORES)), **kw)


def kernel(x_norm, xt, qk_w, qk_b, v_fact, out_fact):
    in_maps = make_in_maps(
        np.asarray(x_norm, np.float32), np.asarray(xt, np.float32),
        np.asarray(qk_w, np.float32), np.asarray(qk_b, np.float32),
        np.asarray(v_fact, np.float32), np.asarray(out_fact, np.float32))
    res = run(in_maps)
    out = np.zeros((BT, D), np.float32)
    for c, r in enumerate(res.results):
        p = r["out"].astype(np.float32)
        b0 = 0 if c < 4 else 1
        out[b0 * T:(b0 + 1) * T] += p[0:T]
        out[(1 - b0) * T:(2 - b0) * T] += p[T:BT]
    return out.reshape(B, T, D)

